# revision 33
# baseline (speedup 1.0000x reference)
"""Trainium2 Bass kernel for nn_FLASH_ShareA_FFConvM (v2: fp8 + deep pipeline).

Data-parallel over (batch, seq-half): 8 cores x 4096 tokens (16 local-attention
chunks of 256). Weights replicated, pre-packed fp8 (DoubleRow pairs) on host.

Per core:
  Phase A (merged, single x load per tile): LN stats (bn_stats/aggr), rstd via
    DVE Newton-rsqrt (no ACT Sqrt => ACT uses only Exp/Silu tables), normalized
    xs -> bf16 token-major (persisted residual tiles incl. token-shift halves)
    -> DMA-transpose -> channel-major xs8 (fp8, x8 scale) with the token-shift
    of channels 0..255 expressed as a column offset.
  Pair-pipelined main loop (pair = 2 chunks = 512 tokens), stages lagged:
    FRONT(p):  qk/gate/hidden/vgate DoubleRow matmuls + evacuations
    SIM(p-1):  per-chunk OffsetScale stats, q/k/qs/ks, sim matmuls with the
               causal mask added via an identity matmul (fully-masked 128x128
               block skipped), Exp from PSUM (scale folded), ones-matmul
               denominators
    TAIL(p-2): reciprocal + DRAM-bounce broadcast, attn = exp*rec (GpSimd,
               fp8 out), V matmuls (DoubleRow over the two key tiles),
               silu (ACT), og = silu*gate (GpSimd, fp8)
    FIN(p-3):  fin = xs@wcomb + og@w_out[512:] (DoubleRow), y = fin*silu(vgate)
               + residual, DMA out.
  ACT runs only {Silu, Exp, Copy}; per iteration the chained order
  [silu batch][exp batch] costs exactly 2 table loads.
"""

import sys

sys.path.insert(0, "/opt/trn_rl_repo")

import numpy as np
import ml_dtypes
from contextlib import ExitStack

import concourse.bass as bass
import concourse.tile as tile
from concourse import bacc, mybir

F32 = mybir.dt.float32
BF16 = mybir.dt.bfloat16
FP8 = mybir.dt.float8e4
I32 = mybir.dt.int32
AX = mybir.AxisListType
ALU = mybir.AluOpType
ACTF = mybir.ActivationFunctionType
PM = mybir.MatmulPerfMode

B, SEQ, DIM = 4, 8192, 512
G, QK = 32, 128
CHUNK = SEQ // G          # 256 tokens per attention chunk
HD = QK // 4              # 32 (softmax scale dim)
SCALE = float(HD) ** -0.5
HID = DIM
EPS = 1e-5
N_CORES = 8
T_CORE = SEQ // 2         # 4096 tokens per core
NEGM = -20000.0           # mask add; exp(SCALE*NEGM) == 0

BF = ml_dtypes.bfloat16
E4 = ml_dtypes.float8_e4m3

# fp8 scale plan (see module docstring):
#   xs8 = 8*xs ; w*8 = 64*w (qk/hvg/ga) ; wcomb8 = 256*wcomb ; woa8 = 128*woa
#   front psums = 512*true ; hid8 = 16*hid ; gate16 = 16*gate
#   denominator ones = 1/64 -> rec = 64/den ; attn8 = 64*attn
#   vps = 1024*out -> silu scale 1/1024 ; og8 = 16*og ; fin psum = 2048*fin
S_X = 8.0
S_W = 64.0
S_WC = 256.0
S_WOA = 128.0
MAGIC = 0x5F3759DF


def build_core_program(ctx, tc, aps, n_tok, apply_g, apply_b):
    nc = tc.nc
    n_tiles = n_tok // 128            # 32 (excl. halo)
    n_chunks = n_tok // CHUNK         # 16
    n_pairs = n_chunks // 2           # 8
    n_pad = n_tok + 128
    n_atiles = n_tiles + 1            # 33 incl. halo

    xp = aps["xp"]
    yout = aps["y"]

    consts = ctx.enter_context(tc.tile_pool(name="consts", bufs=1))
    persist = ctx.enter_context(tc.tile_pool(name="persist", bufs=1))
    work = ctx.enter_context(tc.tile_pool(name="work", bufs=1))
    psum = ctx.enter_context(tc.tile_pool(name="psum", bufs=1, space="PSUM"))
    dram = ctx.enter_context(tc.tile_pool(name="dram", bufs=1, space="DRAM"))

    def cload(name, shape, dtype):
        t = consts.tile(shape, dtype, name=f"c_{name}", tag=f"c_{name}")
        nc.sync.dma_start(t[:], aps[name])
        return t

    wqk8 = cload("wqk8", [128, 4, 128], FP8)
    wga8 = cload("wga8", [128, 4, 512], FP8)
    whvg8 = cload("whvg8", [128, 4, 1024], FP8)
    wcomb8 = cload("wcomb8", [128, 4, 512], FP8)
    woa8 = cload("woa8", [128, 8, 2, 512], FP8)
    g46 = cload("g46", [128, 6], F32)
    b46 = cload("b46", [128, 6], F32)
    maskd4 = cload("maskd4", [128, 512], BF16)
    identb = cload("identb", [128, 128], BF16)
    if apply_g:
        lng = cload("lng", [128, 512], F32)
    if apply_b:
        lnb = cload("lnb", [128, 512], F32)

    ones64 = consts.tile([128, 1], BF16, name="ones64", tag="ones64")
    nc.vector.memset(ones64[:], 1.0 / 64.0)
    magict = consts.tile([128, 12], I32, name="magict", tag="magict")
    nc.vector.memset(magict[:], MAGIC)

    # zero-padded per-head key tiles (K=128 sim matmuls without row-group
    # tiling: concurrent row-group MMs draining into one PSUM bank are fatal).
    # [:, h, :] holds head h's 64 qk dims at partitions h*64..h*64+63, rest 0.
    kz_ring = [consts.tile([128, 2, 256], BF16, name=f"kz{s}", tag=f"kz{s}")
               for s in range(2)]
    ksz_ring = [consts.tile([128, 2, 256], BF16, name=f"ksz{s}",
                            tag=f"ksz{s}") for s in range(2)]
    for t in (*kz_ring, *ksz_ring):
        nc.vector.memset(t[:], 0.0)

    # persistent activations; channels 0..255 stored pre-shifted by one
    # token (written at col+1) so all matmul slices share one aligned offset
    xsT8 = persist.tile([128, 4, n_pad + 16], FP8, name="xsT8", tag="xsT8")
    resid = persist.tile([128, n_tiles, 512], BF16, name="resid", tag="resid")

    def act(bi):
        tc.chain_iter_dep("actfn", bi.ins)
        return bi

    # ------------------------------------------------------------------
    # phase A: merged LN (one x load), rsqrt on DVE, fp8 channel-major xs
    # ------------------------------------------------------------------
    prev_xs = [None]          # bf16 token-major tile of previous A-tile

    A_BATCH = 2

    def emit_A_batch(b):
        i0 = b * A_BATCH
        nb = min(A_BATCH, n_atiles - i0)
        if nb <= 0:
            return
        xts = []
        mvb = work.tile([128, 12], F32, name=f"mvb{b}", tag="mvb", bufs=2)
        for j in range(nb):
            i = i0 + j
            x_t = work.tile([128, 512], F32, name=f"xa{i}", tag="xa", bufs=2)
            nc.sync.dma_start(x_t[:], xp[i * 128:(i + 1) * 128, :])
            xts.append(x_t)
            bns = work.tile([128, 6], F32, name=f"bns{i}", tag="bns", bufs=2)
            nc.vector.bn_stats(out=bns[:], in_=x_t[:])
            nc.vector.bn_aggr(out=mvb[:, 2 * j:2 * j + 2], in_=bns[:])
        # Newton rsqrt over the whole [128, 2*nb] (mean cols give junk,
        # ignored); y = 1/sqrt(v + eps)
        w_ = 2 * nb
        vt = work.tile([128, 12], F32, name=f"vt{b}", tag="vt", bufs=2)
        nc.vector.tensor_scalar_add(out=vt[:, :w_], in0=mvb[:, :w_],
                                    scalar1=EPS)
        sh = work.tile([128, 12], I32, name=f"sh{b}", tag="sh", bufs=2)
        nc.vector.tensor_scalar(out=sh[:, :w_], in0=vt[:, :w_].bitcast(I32),
                                scalar1=1, scalar2=None,
                                op0=ALU.arith_shift_right)
        y0i = work.tile([128, 12], I32, name=f"y0i{b}", tag="y0i", bufs=2)
        nc.vector.tensor_sub(out=y0i[:, :w_], in0=magict[:, :w_],
                             in1=sh[:, :w_])
        yy = work.tile([128, 12], F32, name=f"yy{b}", tag="yy", bufs=2)
        tb = work.tile([128, 12], F32, name=f"tb{b}", tag="tb", bufs=2)
        nc.vector.tensor_copy(out=yy[:, :w_], in_=y0i[:, :w_].bitcast(F32))
        for _ in range(2):
            nc.vector.tensor_mul(out=tb[:, :w_], in0=yy[:, :w_],
                                 in1=yy[:, :w_])
            nc.vector.tensor_mul(out=tb[:, :w_], in0=tb[:, :w_],
                                 in1=vt[:, :w_])
            nc.vector.tensor_scalar(out=tb[:, :w_], in0=tb[:, :w_],
                                    scalar1=-0.5, scalar2=1.5, op0=ALU.mult,
                                    op1=ALU.add)
            nc.vector.tensor_mul(out=yy[:, :w_], in0=yy[:, :w_],
                                 in1=tb[:, :w_])
        for j in range(nb):
            i = i0 + j
            xs_bf = work.tile([128, 512], BF16, name=f"xsbf{i}", tag="xsbf",
                              bufs=3)
            mean = mvb[:, 2 * j:2 * j + 1]
            rstd = yy[:, 2 * j + 1:2 * j + 2]
            if apply_g or apply_b:
                xf = work.tile([128, 512], F32, name=f"xf{i}", tag="xf",
                               bufs=2)
                nc.vector.tensor_scalar(out=xf[:], in0=xts[j][:],
                                        scalar1=mean, scalar2=rstd,
                                        op0=ALU.subtract, op1=ALU.mult)
                if apply_g:
                    nc.vector.tensor_mul(out=xf[:], in0=xf[:], in1=lng[:])
                if apply_b:
                    nc.vector.tensor_add(out=xf[:], in0=xf[:], in1=lnb[:])
                nc.vector.tensor_copy(out=xs_bf[:], in_=xf[:])
            else:
                nc.vector.tensor_scalar(out=xs_bf[:], in0=xts[j][:],
                                        scalar1=mean, scalar2=rstd,
                                        op0=ALU.subtract, op1=ALU.mult)
            # channel-major fp8 (x8) via bf16 transpose slab
            slab = work.tile([128, 4, 128], BF16, name=f"slab{i}", tag="slab",
                             bufs=2)
            nc.sync.dma_start(slab[:], xs_bf[:], transpose=True)
            nc.scalar.mul(out=xsT8[:, 0:2, i * 128 + 1:(i + 1) * 128 + 1],
                          in_=slab[:, 0:2, :], mul=S_X)
            nc.vector.tensor_scalar_mul(
                out=xsT8[:, 2:4, i * 128:(i + 1) * 128],
                in0=slab[:, 2:4, :], scalar1=S_X)
            if i >= 1:
                ti = i - 1
                nc.sync.dma_start(resid[1:128, ti, 0:256],
                                  xs_bf[0:127, 0:256])
                nc.sync.dma_start(resid[0:1, ti, 0:256],
                                  prev_xs[0][127:128, 0:256])
                nc.vector.tensor_copy(out=resid[:, ti, 256:512],
                                      in_=xs_bf[:, 256:512])
            prev_xs[0] = xs_bf

    def xsh8(cp, col, width):
        """DoubleRow slice of xsT8 for channel pair cp (the token shift of
        channels 0..255 is baked into the storage offset)."""
        return xsT8[:, 2 * cp:2 * cp + 2, col:col + width]

    # ------------------------------------------------------------------
    # main pipeline state (rings keyed by tags)
    # ------------------------------------------------------------------
    def qkT_t(p):
        return work.tile([128, 512], BF16, name=f"qkT{p}", tag="qkT", bufs=2)

    qkT_ring = {}
    gate_ring = {}
    hid_ring = {}
    sv_ring = {}
    expt_ring = {}
    attn_ring = {}
    og_ring = {}
    den_ring = {}
    recb_ring = {}
    osl_ring = {}

    # ---------------- FRONT ----------------
    def emit_front_qkgate(p):
        colP = 128 + p * 512
        qkps = psum.tile([128, 512], F32, name=f"qkps{p}", tag="fr", bufs=2)
        for cp in range(2):
            nc.tensor.matmul(qkps[:], wqk8[:, 2 * cp:2 * cp + 2, :],
                             xsh8(cp, colP, 512), start=(cp == 0),
                             stop=(cp == 1), perf_mode=PM.DoubleRow)
        qkT = qkT_t(p)
        qkT_ring[p] = qkT
        nc.scalar.mul(out=qkT[:], in_=qkps[:], mul=1.0 / (S_X * S_W))
        gts = []
        for ee in range(4):
            gps = psum.tile([128, 512], F32, name=f"g{p}_{ee}", tag="fr",
                            bufs=2)
            for cp in range(2):
                nc.tensor.matmul(gps[:],
                                 wga8[:, 2 * cp:2 * cp + 2,
                                      ee * 128:(ee + 1) * 128],
                                 xsh8(cp, colP, 512), start=(cp == 0),
                                 stop=(cp == 1), perf_mode=PM.DoubleRow)
            gb = work.tile([128, 512], BF16, name=f"g16_{p}_{ee}",
                           tag=f"g16{ee}", bufs=4)
            nc.vector.tensor_scalar_mul(out=gb[:], in0=gps[:],
                                        scalar1=16.0 / (S_X * S_W))
            gts.append(gb)
        gate_ring[p] = gts

    def emit_front_hv(p):
        hts = [
            work.tile([128, 2, 512], FP8, name=f"hid8_{p}_{c}", tag="hid8",
                      bufs=8) for c in range(2)
        ]
        hid_ring[p] = hts
        vgs = []
        for tt in range(4):
            ti = 4 * p + tt
            colT = 128 + ti * 128
            hps = psum.tile([128, 512], F32, name=f"h{p}_{tt}", tag="mix",
                            bufs=3)
            for cp in range(2):
                nc.tensor.matmul(hps[:], xsh8(cp, colT, 128),
                                 whvg8[:, 2 * cp:2 * cp + 2, 0:512],
                                 start=(cp == 0), stop=(cp == 1),
                                 perf_mode=PM.DoubleRow)
            vps2 = psum.tile([128, 512], F32, name=f"v{p}_{tt}", tag="mix",
                             bufs=3)
            for cp in range(2):
                nc.tensor.matmul(vps2[:], xsh8(cp, colT, 128),
                                 whvg8[:, 2 * cp:2 * cp + 2, 512:1024],
                                 start=(cp == 0), stop=(cp == 1),
                                 perf_mode=PM.DoubleRow)
            nc.vector.tensor_scalar_mul(out=hts[tt // 2][:, tt % 2, :],
                                        in0=hps[:],
                                        scalar1=16.0 / (S_X * S_W))
            vg = work.tile([128, 512], BF16, name=f"vg{p}_{tt}", tag="vg",
                           bufs=8)
            nc.scalar.mul(out=vg[:], in_=vps2[:], mul=1.0 / (S_X * S_W))
            vgs.append(vg)
        sv_ring[p] = ("raw", vgs)

    # ---------------- SIM (pair p-1) ----------------
    def emit_qstat_sim(g):
        p = g // 2
        half = g % 2
        qk_c = qkT_ring[p][:, half * 256:(half + 1) * 256]
        qsum = work.tile([128, 1], F32, name=f"qsum{g}", tag="qsum", bufs=2)
        nc.vector.tensor_reduce(out=qsum[:], in_=qk_c, axis=AX.X, op=ALU.add)
        offs = work.tile([128, 6], F32, name=f"offs{g}", tag="offs", bufs=2)
        nc.vector.scalar_tensor_tensor(out=offs[:], in0=g46[:],
                                       scalar=qsum[:], in1=b46[:],
                                       op0=ALU.mult, op1=ALU.add)
        qoff, koff, qsc, ksc, qsoff, ksoff = (offs[:, i:i + 1]
                                              for i in range(6))
        qT = work.tile([128, 256], BF16, name=f"qT{g}", tag="qT", bufs=2)
        kT = work.tile([128, 256], BF16, name=f"kT{g}", tag="kT", bufs=2)
        nc.vector.tensor_scalar(out=qT[:], in0=qk_c, scalar1=qsc,
                                scalar2=qoff, op0=ALU.mult, op1=ALU.add)
        nc.vector.tensor_scalar(out=kT[:], in0=qk_c, scalar1=ksc,
                                scalar2=koff, op0=ALU.mult, op1=ALU.add)
        qsT = work.tile([128, 256], BF16, name=f"qsT{g}", tag="qsT", bufs=2)
        ksT = work.tile([128, 256], BF16, name=f"ksT{g}", tag="ksT", bufs=2)
        nc.vector.tensor_copy(out=qsT[:, 0:1], in_=qsoff)
        nc.vector.tensor_copy(out=ksT[:, 0:1], in_=ksoff)
        nc.vector.tensor_scalar(out=qsT[:, 1:256], in0=qT[:, 0:255],
                                scalar1=qsc, scalar2=qsoff, op0=ALU.mult,
                                op1=ALU.add)
        nc.vector.tensor_scalar(out=ksT[:, 1:256], in0=kT[:, 0:255],
                                scalar1=ksc, scalar2=ksoff, op0=ALU.mult,
                                op1=ALU.add)
        kz = kz_ring[g % 2]
        ksz = ksz_ring[g % 2]
        nc.vector.tensor_copy(out=kz[0:64, 0, :], in_=kT[0:64, :])
        nc.vector.tensor_copy(out=kz[64:128, 1, :], in_=kT[64:128, :])
        nc.vector.tensor_copy(out=ksz[0:64, 0, :], in_=ksT[0:64, :])
        nc.vector.tensor_copy(out=ksz[64:128, 1, :], in_=ksT[64:128, :])

        # sim psum tiles: [j, (h, i128)]
        simA0 = psum.tile([128, 512], F32, name=f"sA0{g}", tag="sm", bufs=2)
        simA1 = psum.tile([128, 512], F32, name=f"sA1{g}", tag="sm", bufs=2)
        simB = psum.tile([128, 512], F32, name=f"sB{g}", tag="sm", bufs=2)
        QKp = [(qT, kz), (qsT, ksz)]

        def blk(out_t, jt, it, stop_last):
            for hp in range(2):
                Q, K = QKp[hp]
                for hh in range(2):
                    h = hp * 2 + hh
                    nc.tensor.matmul(
                        out_t[:, h * 128:(h + 1) * 128],
                        K[:, hh, jt * 128:jt * 128 + 128],
                        Q[:, it * 128:it * 128 + 128],
                        start=(h == 0), stop=(stop_last and h == 3),
                        skip_group_check=True)

        # (jt0,it0): diagonal block, needs the causal mask
        blk(simA0, 0, 0, False)
        nc.tensor.matmul(simA0[:], identb[:], maskd4[:], start=False,
                         stop=True, skip_group_check=True)
        # (jt0,it1): every key precedes every query -> unmasked
        blk(simA1, 0, 1, True)
        # (jt1,it1): diagonal block
        blk(simB, 1, 1, False)
        nc.tensor.matmul(simB[:], identb[:], maskd4[:], start=False,
                         stop=True, skip_group_check=True)
        return simA0, simA1, simB

    def emit_exp(g, sims):
        simA0, simA1, simB = sims
        e0 = work.tile([128, 512], BF16, name=f"e0{g}", tag="e0", bufs=3)
        e1 = work.tile([128, 512], BF16, name=f"e1{g}", tag="e1", bufs=3)
        e2 = work.tile([128, 512], BF16, name=f"e2{g}", tag="e2", bufs=3)
        act(nc.scalar.activation(out=e0[:], in_=simA0[:], func=ACTF.Exp,
                                 scale=SCALE))
        act(nc.scalar.activation(out=e1[:], in_=simA1[:], func=ACTF.Exp,
                                 scale=SCALE))
        act(nc.scalar.activation(out=e2[:], in_=simB[:], func=ACTF.Exp,
                                 scale=SCALE))
        expt_ring[g] = (e0, e1, e2)

    def emit_denom(g):
        e0, e1, e2 = expt_ring[g]
        d0 = psum.tile([1, 512], F32, name=f"d0{g}", tag="sm", bufs=2)
        d1 = psum.tile([1, 512], F32, name=f"d1{g}", tag="sm", bufs=2)
        nc.tensor.matmul(d0[0:1, :], ones64[:], e0[:], start=True, stop=True)
        nc.tensor.matmul(d1[0:1, :], ones64[:], e1[:], start=True, stop=False)
        nc.tensor.matmul(d1[0:1, :], ones64[:], e2[:], start=False, stop=True)
        return d0, d1

    def emit_dencopy(g, dd):
        den_ring[g] = dd

    # ---------------- TAIL (pair p-2) ----------------
    def emit_tail_rec(g):
        d0, d1 = den_ring.pop(g)
        recf = work.tile([1, 1024], F32, name=f"rcf{g}", tag="rcf", bufs=2)
        nc.vector.reciprocal_approx_fast(out=recf[:, 0:512], in_=d0[0:1, :])
        nc.vector.reciprocal_approx_fast(out=recf[:, 512:1024],
                                         in_=d1[0:1, :])
        recb_bf = work.tile([1, 1024], BF16, name=f"rcb{g}", tag="rcb",
                            bufs=2)
        nc.vector.tensor_copy(out=recb_bf[:], in_=recf[:])
        rd = dram.tile([1, 1024], BF16, name=f"rd{g}", tag="rd", bufs=2)
        nc.sync.dma_start(rd[:], recb_bf[:])
        recb = work.tile([128, 1024], BF16, name=f"recb{g}", tag="recb",
                         bufs=2)
        bcast = bass.AP(tensor=rd.tensor, offset=rd.offset,
                        ap=[[0, 128], [1, 1024]])
        nc.sync.dma_start(recb[:], bcast)
        recb_ring[g] = recb

    def emit_tail_attn(g):
        e0, e1, e2 = expt_ring[g]
        recb = recb_ring[g]
        a8 = work.tile([128, 2, 1024], FP8, name=f"a8{g}", tag="a8", bufs=4)
        nc.gpsimd.tensor_mul(out=a8[:, 0, 0:512], in0=e0[:],
                             in1=recb[:, 0:512])
        nc.gpsimd.tensor_mul(out=a8[:, 0, 512:1024], in0=e1[:],
                             in1=recb[:, 512:1024])
        nc.gpsimd.tensor_mul(out=a8[:, 1, 512:1024], in0=e2[:],
                             in1=recb[:, 512:1024])
        attn_ring[g] = a8

    def emit_V(g):
        p = g // 2
        a8 = attn_ring[g]
        hid8 = hid_ring[p][g % 2]
        vtiles = []
        for ee in range(4):
            v0 = psum.tile([128, 512], F32, name=f"v0{g}_{ee}", tag="mix",
                           bufs=3)
            nc.tensor.matmul(v0[:], hid8[:, 0, ee * 128:(ee + 1) * 128],
                             a8[:, 0, 0:512], start=True, stop=True)
            v1 = psum.tile([128, 512], F32, name=f"v1{g}_{ee}", tag="mix",
                           bufs=3)
            nc.tensor.matmul(v1[:], hid8[:, :, ee * 128:(ee + 1) * 128],
                             a8[:, :, 512:1024], start=True, stop=True,
                             perf_mode=PM.DoubleRow)
            vtiles.append((v0, v1))
        return vtiles

    def emit_osl(g, vtiles):
        osls = []
        for ee in range(4):
            v0, v1 = vtiles[ee]
            o0 = work.tile([128, 512], FP8, name=f"o0{g}_{ee}",
                           tag=f"osl0{ee}", bufs=2)
            o1 = work.tile([128, 512], FP8, name=f"o1{g}_{ee}",
                           tag=f"osl1{ee}", bufs=2)
            act(nc.scalar.activation(out=o0[:], in_=v0[:], func=ACTF.Silu,
                                     scale=1.0 / 1024.0))
            act(nc.scalar.activation(out=o1[:], in_=v1[:], func=ACTF.Silu,
                                     scale=1.0 / 1024.0))
            osls.append((o0, o1))
        osl_ring[g] = osls

    def emit_sv(p):
        kind, vgs = sv_ring[p]
        svs = []
        for tt in range(4):
            sv = work.tile([128, 512], BF16, name=f"sv{p}_{tt}", tag="sv",
                           bufs=16)
            act(nc.scalar.activation(out=sv[:], in_=vgs[tt][:],
                                     func=ACTF.Silu))
            svs.append(sv)
        sv_ring[p] = ("silu", svs)

    def emit_og(g):
        p = g // 2
        half = g % 2
        osls = osl_ring[g]
        og8 = work.tile([128, 4, 1024], FP8, name=f"og8{g}", tag="og8",
                        bufs=2)
        for ee in range(4):
            gslice = gate_ring[p][ee]
            for it in range(2):
                gbc = gslice[:, half * 256 + it * 128:
                             half * 256 + it * 128 + 128]
                gbc = gbc.unsqueeze(1).broadcast_to((128, 4, 128))
                nc.gpsimd.tensor_tensor(
                    out=og8[:, ee, it * 512:(it + 1) * 512].rearrange(
                        "p (h i) -> p h i", h=4),
                    in0=osls[ee][it][:].rearrange("p (h i) -> p h i", h=4),
                    in1=gbc, op=ALU.mult)
        og_ring[g] = og8

    # ---------------- FIN (pair p-3) ----------------
    def emit_fin_tt(ti):
        p = ti // 4
        g = ti // 2
        it = ti % 2
        colT = 128 + ti * 128
        og8 = og_ring[g]
        fin = psum.tile([128, 512], F32, name=f"fin{ti}", tag="fin", bufs=1)
        for cp in range(2):
            nc.tensor.matmul(fin[:], xsh8(cp, colT, 128),
                             wcomb8[:, 2 * cp:2 * cp + 2, :],
                             start=(cp == 0), stop=False,
                             perf_mode=PM.DoubleRow)
        for h in range(4):
            for ep in range(2):
                nc.tensor.matmul(
                    fin[:],
                    og8[:, 2 * ep:2 * ep + 2,
                        it * 512 + h * 128:it * 512 + h * 128 + 128],
                    woa8[:, h * 2 + ep, :, :],
                    start=False, stop=(h == 3 and ep == 1),
                    perf_mode=PM.DoubleRow)
        sv = sv_ring[p][1][ti % 4]
        y1 = work.tile([128, 512], BF16, name=f"y1_{ti}", tag="y1", bufs=2)
        # y = (fin * 1/2048) * sv  (fin psum = 2048*true; og/woa/xs scales)
        nc.vector.scalar_tensor_tensor(out=y1[:], in0=fin[:],
                                       scalar=1.0 / 2048.0, in1=sv[:],
                                       op0=ALU.mult, op1=ALU.mult)
        y = work.tile([128, 512], F32, name=f"y{ti}", tag="y", bufs=2)
        nc.vector.tensor_add(out=y[:], in0=y1[:], in1=resid[:, ti, :])
        nc.sync.dma_start(yout[ti * 128:(ti + 1) * 128, :], y[:])

    # ------------------------------------------------------------------
    # prologue + pipelined loop
    # ------------------------------------------------------------------
    emit_A_batch(0)
    emit_A_batch(1)
    emit_A_batch(2)

    n_batches = (n_atiles + A_BATCH - 1) // A_BATCH
    next_batch = [3]
    sims_pend = {}
    den_pend = {}
    vt_pend = {}

    for i in range(n_pairs + 4):
        pF, pS, pV, pW, pY = i, i - 1, i - 2, i - 3, i - 4
        if pF < n_pairs:
            emit_front_qkgate(pF)
        if 0 <= pV < n_pairs:
            for g in (2 * pV, 2 * pV + 1):
                emit_tail_rec(g)
        if pF < n_pairs:
            emit_front_hv(pF)
        if 0 <= pV < n_pairs:
            for g in (2 * pV, 2 * pV + 1):
                emit_tail_attn(g)
        if 0 <= pS < n_pairs:
            for g in (2 * pS, 2 * pS + 1):
                sims_pend[g] = emit_qstat_sim(g)
        # ACT chained batches: [sv(pS)] [exp(pS)] [osl(pW)] -> 2 table loads
        # (sv lagged one pair so the ACT queue head never waits on this
        # iteration's front matmuls)
        if 0 <= pS < n_pairs:
            emit_sv(pS)
        if 0 <= pS < n_pairs:
            for g in (2 * pS, 2 * pS + 1):
                emit_exp(g, sims_pend.pop(g))
        if 0 <= pW < n_pairs:
            for g in (2 * pW, 2 * pW + 1):
                vt_pend[g] = emit_V(g)
        if 0 <= pY < n_pairs:
            emit_fin_tt(4 * pY + 0)
            emit_fin_tt(4 * pY + 1)
        if 0 <= pS < n_pairs:
            for g in (2 * pS, 2 * pS + 1):
                den_pend[g] = emit_denom(g)
        if 0 <= pY < n_pairs:
            emit_fin_tt(4 * pY + 2)
            emit_fin_tt(4 * pY + 3)
        if 0 <= pW < n_pairs:
            for g in (2 * pW, 2 * pW + 1):
                emit_osl(g, vt_pend.pop(g))
        if 0 <= pS < n_pairs:
            for g in (2 * pS, 2 * pS + 1):
                emit_dencopy(g, den_pend.pop(g))
        if 0 <= pW < n_pairs:
            for g in (2 * pW, 2 * pW + 1):
                emit_og(g)
        while (pF < n_pairs and next_batch[0] < n_batches
               and next_batch[0] * A_BATCH <= 4 * pF + 14):
            emit_A_batch(next_batch[0])
            next_batch[0] += 1


def make_host_inputs(x, ln_g, ln_b, w_qk, g4, b4, g2, b2, w_hidden, w_gate,
                     w_out, n_tok=T_CORE):
    x = np.asarray(x, np.float32)
    ln_g = np.asarray(ln_g, np.float32)
    ln_b = np.asarray(ln_b, np.float32)
    apply_g = not np.all(ln_g == 1.0)
    apply_b = bool(np.any(ln_b != 0.0))

    w_hidden = np.asarray(w_hidden, np.float32)
    w_out = np.asarray(w_out, np.float32)
    w_gate = np.asarray(w_gate, np.float32)
    w_qk = np.asarray(w_qk, np.float32)
    wcomb = (w_hidden[:, :HID] @ w_out[:HID, :]).astype(np.float32)

    def fold8(w, scale):
        # [512, N] -> [128, 4, N] fp8 with row = cc*128 + p
        w = np.clip(w * scale, -240, 240).astype(E4)
        return w.reshape(4, 128, -1).transpose(1, 0, 2).copy()

    woa = np.clip(w_out[HID:] * S_WOA, -240, 240).astype(E4)  # [2048, 512]
    # [128, 8, 2, 512]: [p, h*2+ep, o, d] = woa[h*512 + (ep*2+o)*128 + p, d]
    woa8 = woa.reshape(4, 2, 2, 128, 512).transpose(3, 0, 1, 2, 4)
    woa8 = woa8.reshape(128, 8, 2, 512).copy()

    jj, ii = np.meshgrid(np.arange(128), np.arange(128), indexing="ij")
    tri = np.where(jj > ii, np.float32(NEGM), np.float32(0.0))
    maskd4 = np.tile(tri, (1, 4)).astype(BF)

    shared = {
        "wqk8": fold8(w_qk, S_W),
        "wga8": fold8(w_hidden[:, HID:], S_W),
        "whvg8": fold8(np.concatenate([w_hidden[:, :HID], w_gate], axis=1),
                       S_W),
        "wcomb8": fold8(wcomb, S_WC),
        "woa8": woa8,
        "g46": np.concatenate(
            [(np.asarray(g4, np.float32) / CHUNK).T,
             (np.asarray(g2, np.float32) / CHUNK).T], axis=1).copy(),
        "b46": np.concatenate(
            [np.asarray(b4, np.float32).T,
             np.asarray(b2, np.float32).T], axis=1).copy(),
        "maskd4": maskd4,
        "identb": np.eye(128, dtype=np.float32).astype(BF),
    }
    if apply_g:
        shared["lng"] = np.broadcast_to(ln_g, (128, DIM)).copy()
    if apply_b:
        shared["lnb"] = np.broadcast_to(ln_b, (128, DIM)).copy()

    n_half = x.shape[1] // n_tok
    per_core = []
    for core in range(x.shape[0] * n_half):
        b = core // n_half
        h = core % n_half
        t0 = h * n_tok
        xp = np.zeros((n_tok + 128, DIM), np.float32)
        xp[128:] = x[b, t0:t0 + n_tok]
        if t0 > 0:
            xp[127] = x[b, t0 - 1]
        per_core.append({"xp": xp})
    return shared, per_core, apply_g, apply_b


def build_bass(n_tok, apply_g, apply_b):
    nc = bacc.Bacc("TRN2", target_bir_lowering=False, debug=False,
                   num_devices=1)
    specs = {
        "xp": ([n_tok + 128, DIM], F32),
        "wqk8": ([128, 4, 128], FP8),
        "wga8": ([128, 4, 512], FP8),
        "whvg8": ([128, 4, 1024], FP8),
        "wcomb8": ([128, 4, 512], FP8),
        "woa8": ([128, 8, 2, 512], FP8),
        "g46": ([128, 6], F32),
        "b46": ([128, 6], F32),
        "maskd4": ([128, 512], BF16),
        "identb": ([128, 128], BF16),
    }
    if apply_g:
        specs["lng"] = ([128, 512], F32)
    if apply_b:
        specs["lnb"] = ([128, 512], F32)
    aps = {}
    for name, (shape, dt) in specs.items():
        aps[name] = nc.dram_tensor(name, shape, dt, kind="ExternalInput").ap()
    aps["y"] = nc.dram_tensor("y", [n_tok, DIM], F32,
                              kind="ExternalOutput").ap()

    with tile.TileContext(nc) as tc:
        with ExitStack() as ctx:
            build_core_program(ctx, tc, aps, n_tok, apply_g, apply_b)
    nc.compile()
    return nc


def _run(inputs, trace=False, **spmd_kwargs):
    from concourse.bass_utils import run_bass_kernel_spmd

    shared, per_core, apply_g, apply_b = make_host_inputs(
        inputs["x"], inputs["ln_g"], inputs["ln_b"], inputs["w_qk"],
        inputs["g4"], inputs["b4"], inputs["g2"], inputs["b2"],
        inputs["w_hidden"], inputs["w_gate"], inputs["w_out"])

    nc = build_bass(T_CORE, apply_g, apply_b)

    in_maps = [{**shared, **pc} for pc in per_core]
    res = run_bass_kernel_spmd(nc, in_maps, core_ids=list(range(N_CORES)),
                               trace=trace, **spmd_kwargs)

    y = np.empty((B, SEQ, DIM), np.float32)
    n_half = SEQ // T_CORE
    for core in range(N_CORES):
        b = core // n_half
        h = core % n_half
        y[b, h * T_CORE:(h + 1) * T_CORE] = res.results[core]["y"]
    return y, res


def kernel(**inputs):
    return _run(inputs)[0]
